# revision 21
# baseline (speedup 1.0000x reference)
"""Trainium2 Bass kernel for LoopyBeliefPropagation (3-iter, mask=ones).

Math: for each (b, h) slice define tile[d,s] = s_sib[b,d,h,s],
SP = softplus(tile).  Collapsing the reference's 3-iteration loop into
closed form (see kernel_baseline.py) and folding every
stream-independent term into host constants leaves, per (d,h):

  RS[d,h] = sum_s SP[d,s]        (row-reduce of the stream, DVE/Pool)
  CS[d,h] = sum_s SP[s,d]        (col sums via ones matmuls, PE)
  r1 = CS + C5
  w  = r1 * OME,  S1 = colsum(w) broadcast   (PE matmul)
  bdiff = r1*NF2 + (CS + C7)*NFp2 - RS*NFp1 + w - S1
  out1 = sigmoid(bdiff) = 1/(1+q),  q = exp(-max(bdiff, -30))
  out0 = q * out1

C5, C7, NF2, NFp1, NFp2, OME are [L,H] host constants (they only touch
O(L*H) gathered values, not the 4 MiB stream).  softplus = Ln(Exp(x)+1)
using the natural_log_exp ACT table; the +1 rides the Ln bias.

Device schedule: the s_sib shard streams in ramped h-chunks; Exp/Ln run
back-to-back on ACT (the spine; a dummy Exp right after a memset pulls
the ACT table load to t~0).  The h-columns are independent, so the tail
algebra runs in three h-slices pipelined against the spine; work is
spread over DVE and the otherwise-idle GPSIMD (Pool) engine so only the
last narrow slice (plus its output DMA) trails the spine.

Sharding: 8 cores x (b in 0..3, h-half in {0:64, 64:128}).
"""

import numpy as np

L = 128
H = 64            # h-slices per core
N_CORES = 8
LN2 = float(np.log(2.0))

# ramped chunk sizes; tails cover h [0:32], [32:58], [58:64]
CHUNKS = [2, 6, 10, 16, 14, 10, 4, 2]
TAILS = [(0, 34, 3), (34, 24, 5), (58, 6, 7)]    # (h0, width, after-chunk)
RED_ENG = ["v", "v", "v", "v", "v", "v", "v", "v"]   # per-chunk row-reduce
CHAIN_ENG = ["p", "p", "v"]                          # per-tail algebra
assert sum(CHUNKS) == H

# host-constant column layout: 5 tensors of H columns each
C_M1 = 0 * H
C_M0 = 1 * H
C_M2 = 2 * H
C_OME = 3 * H
C_NFP1 = 4 * H
C_COLS = 5 * H

_PROGRAM = None


def _build_program():
    import concourse.bacc as bacc
    import concourse.mybir as mybir
    import concourse.tile as tile

    fp32 = mybir.dt.float32
    AF = mybir.ActivationFunctionType
    OP = mybir.AluOpType

    # Exp and Ln live in one PWP table; without this filter the table
    # chooser maps Exp to exp_and_others and Ln to natural_log_exp_and_
    # others and reloads the ACT table (~1.3us) between every pair.
    if not getattr(bacc, "_lbp_act_tables_patched", False):
        _orig_tables = bacc.get_activation_tables

        def _ln_exp_only(arch):
            t = _orig_tables(arch)
            exp_ln = {AF.Exp, AF.Ln}
            return {
                name: (funcs if name == "natural_log_exp_and_others"
                       else set(funcs) - exp_ln)
                for name, funcs in t.items()
            }

        bacc.get_activation_tables = _ln_exp_only
        bacc._lbp_act_tables_patched = True

    nc = bacc.Bacc(None, target_bir_lowering=False)

    t_d = nc.dram_tensor("t", [L, H, L], fp32, kind="ExternalInput")
    hx_d = nc.dram_tensor("hx", [L, C_COLS], fp32, kind="ExternalInput")
    o_d = nc.dram_tensor("o", [L, H, 2], fp32, kind="ExternalOutput")

    with tile.TileContext(nc) as tc:
        with (
            tc.tile_pool(name="work", bufs=1) as wp,
            tc.tile_pool(name="psum", bufs=1, space="PSUM") as pp,
        ):
            zb = wp.tile([L, 1], fp32, tag="zb")
            ob = wp.tile([L, 1], fp32, tag="ob")
            ones = wp.tile([L, L], fp32, tag="ones")
            nc.gpsimd.memset(zb[:], 0.0)
            nc.gpsimd.memset(ob[:], 1.0)
            nc.gpsimd.memset(ones[:], 1.0)

            # dummy table-user: Bacc places the ACT table load right
            # before this, so it runs at t~0 instead of after chunk 0.
            dummy = wp.tile([L, 1], fp32, tag="dummy")
            nc.scalar.activation(dummy[:], zb[:], AF.Exp, bias=zb[:])

            # stream DMAs: chunk 0 first (critical-path opener); host
            # constants ride the queue behind chunk 3.
            hx = wp.tile([L, C_COLS], fp32, tag="hx")
            tchs = []
            h0 = 0
            for ci, ch in enumerate(CHUNKS):
                tch = wp.tile([L, ch, L], fp32, tag=f"tch{ci}")
                nc.sync.dma_start(tch[:], t_d[:, h0:h0 + ch, :])
                tchs.append((tch, h0, ch))
                if ci == 3:
                    nc.sync.dma_start(hx[:], hx_d[:])
                h0 += ch

            # per-tail state; tails 1 and 2 share merged exp/finals state
            BW = TAILS[1][1] + TAILS[2][1]
            bdcB = wp.tile([L, BW], fp32, tag="bdcB")
            qB = wp.tile([L, BW], fp32, tag="qB")
            sB = wp.tile([L, BW], fp32, tag="sB")
            osbB = wp.tile([L, BW, 2], fp32, tag="osbB")
            tails = []
            for ti, (th0, tw, _) in enumerate(TAILS):
                tails.append({
                    "RS": wp.tile([L, tw], fp32, tag=f"RS{ti}", name=f"RS{ti}"),
                    "cs": pp.tile([L, tw], fp32, tag=f"cs{ti}", name=f"cs{ti}"),
                    "bc": pp.tile([L, tw], fp32, tag=f"bc{ti}", name=f"bc{ti}"),
                    "osb": wp.tile([L, tw, 2], fp32, tag=f"osb{ti}",
                                   name=f"osb{ti}"),
                })
            tails[1]["bdcm"] = bdcB[:, 0:TAILS[1][1]]
            tails[2]["bdcm"] = bdcB[:, TAILS[1][1]:BW]

            def tail_of(h):
                for ti, (th0, tw, _) in enumerate(TAILS):
                    if th0 <= h < th0 + tw:
                        return ti, h - th0
                raise AssertionError

            def hxs(base, ti):
                th0, tw, _ = TAILS[ti]
                return hx[:, base + th0:base + th0 + tw]

            def tail_chain_pre(ti, eng, cs_src):
                """Algebra for tail slice ti up to bd-partial (pre -S1).

                cs_src: SBUF copy of CS for Pool (it cannot read PSUM);
                the PSUM-reading finish lives in tail_chain_fin.
                """
                th0, tw, _ = TAILS[ti]
                T = tails[ti]
                w_ = wp.tile([L, tw], fp32, tag=f"w_{ti}", name=f"w_{ti}")
                rsn = wp.tile([L, tw], fp32, tag=f"rsn_{ti}", name=f"rsn_{ti}")
                bd = wp.tile([L, tw], fp32, tag=f"bd_{ti}", name=f"bd_{ti}")
                T["bd"] = bd
                T["bdc"] = wp.tile([L, tw], fp32, tag=f"bdc_{ti}",
                                   name=f"bdc_{ti}")
                T["q"] = wp.tile([L, tw], fp32, tag=f"q_{ti}", name=f"q_{ti}")
                T["s"] = wp.tile([L, tw], fp32, tag=f"s_{ti}", name=f"s_{ti}")
                eng.tensor_mul(w_[:], cs_src[:], hxs(C_OME, ti))
                eng.tensor_add(w_[:], w_[:], hxs(C_M2, ti))
                nc.tensor.matmul(T["bc"][:], ones[:], w_[:],
                                 start=True, stop=True)
                eng.tensor_mul(bd[:], cs_src[:], hxs(C_M1, ti))
                eng.tensor_add(bd[:], bd[:], hxs(C_M0, ti))
                eng.tensor_mul(rsn[:], T["RS"][:], hxs(C_NFP1, ti))
                eng.tensor_sub(bd[:], bd[:], rsn[:])

            def tail_chain_fin(ti):
                """DVE finish: -S1 (PSUM read) and the exp-domain clamp."""
                T = tails[ti]
                dst = T["bdcm"] if "bdcm" in T else T["bdc"][:]
                nc.vector.tensor_sub(dst, T["bd"][:], T["bc"][:])
                nc.vector.tensor_scalar_max(dst, dst, -30.0)

            def tail_chain(ti, eng):
                if eng is nc.vector:
                    tail_chain_pre(ti, eng, tails[ti]["cs"])
                else:
                    th0, tw, _ = TAILS[ti]
                    cs_s = wp.tile([L, tw], fp32, tag=f"cs_s{ti}",
                                   name=f"cs_s{ti}")
                    nc.vector.tensor_copy(cs_s[:], tails[ti]["cs"][:])
                    tail_chain_pre(ti, eng, cs_s)

            def tail_exp(ti):
                T = tails[ti]
                nc.scalar.activation(T["q"][:], T["bdc"][:], AF.Exp,
                                     bias=zb[:], scale=-1.0)

            def tail_final(ti):
                T = tails[ti]
                nc.vector.tensor_scalar_add(T["s"][:], T["q"][:], 1.0)
                nc.vector.reciprocal(T["osb"][:, :, 1], T["s"][:])
                nc.vector.tensor_mul(T["osb"][:, :, 0], T["q"][:],
                                     T["osb"][:, :, 1])

            def tail_out(ti):
                th0, tw, _ = TAILS[ti]
                nc.sync.dma_start(o_d[:, th0:th0 + tw, :], tails[ti]["osb"][:])

            # spine: exp -> ln(+1) per chunk; row sums + col sums trail
            for ci, (tch, h0, ch) in enumerate(tchs):
                sp = wp.tile([L, ch, L], fp32, tag=f"sp{ci}", name=f"sp{ci}")
                if ci in (5, 6):
                    with tc.tile_wait_until({5: 0.0150, 6: 0.0181}[ci]):
                        nc.scalar.activation(sp[:], tch[:], AF.Exp, bias=zb[:])
                else:
                    nc.scalar.activation(sp[:], tch[:], AF.Exp, bias=zb[:])
                if ci == 6:
                    tail_exp(0)     # expA slots into the ACT queue here
                nc.scalar.activation(sp[:], sp[:], AF.Ln, bias=ob[:])
                ti, off = tail_of(h0)
                assert tail_of(h0 + ch - 1)[0] == ti
                T = tails[ti]
                red = nc.vector if RED_ENG[ci] == "v" else nc.gpsimd
                red.tensor_reduce(
                    T["RS"][:, off:off + ch], sp[:],
                    axis=mybir.AxisListType.X, op=OP.add,
                )
                for j in range(ch):
                    nc.tensor.matmul(
                        T["cs"][:, off + j:off + j + 1],
                        sp[:, j, :],
                        ones[:, 0:1],
                        start=True, stop=True,
                    )
                for ti2, (th0, tw, after) in enumerate(TAILS[:2]):
                    if after == ci:
                        eng = nc.gpsimd if CHAIN_ENG[ti2] == "p" else nc.vector
                        tail_chain(ti2, eng)
                if ci == 4:
                    tail_chain_fin(0)   # DVE: bd_A - S1_A, clamp
                if ci == 5:
                    tail_final(0)
                    tail_out(0)

            tail_chain(2, nc.gpsimd if CHAIN_ENG[2] == "p" else nc.vector)
            tail_chain_fin(2)
            tail_chain_fin(1)
            nc.scalar.activation(qB[:], bdcB[:], AF.Exp, bias=zb[:],
                                 scale=-1.0)
            nc.vector.tensor_scalar_add(sB[:], qB[:], 1.0)
            nc.vector.reciprocal(osbB[:, :, 1], sB[:])
            nc.vector.tensor_mul(osbB[:, :, 0], qB[:], osbB[:, :, 1])
            nc.sync.dma_start(o_d[:, TAILS[1][0]:H, :], osbB[:])

    nc.compile()
    return nc


def _softplus(x):
    return np.logaddexp(0.0, x)


def _core_inputs(s_edge, s_sib, c):
    b, hs = c >> 1, (c & 1) * H
    t = np.ascontiguousarray(s_sib[b, :, hs:hs + H, :], dtype=np.float32)
    d = np.arange(L)
    hl = np.arange(H)

    G = _softplus(t[d[:, None], hl[None, :], (hs + hl)[None, :]])
    DG = _softplus(t[d[:, None], hl[None, :], d[:, None]])
    ROWH = _softplus(
        s_sib[b, (hs + hl)[None, :], (hs + hl)[None, :], d[:, None]])

    E = (d[:, None] == (hs + hl)[None, :]).astype(np.float32)
    NF = 126.0 + E
    CN = LN2 * NF
    OME = 1.0 - E
    PD = (s_edge[b, :, hs:hs + H, 1]
          - s_edge[b, :, hs:hs + H, 0]).astype(np.float32)

    K1 = G + DG - E * G + CN
    K2 = ROWH + DG - E * DG + CN
    PDN = PD * (NF + 1.0)
    S0 = np.sum(PD * OME, axis=0, keepdims=True)   # [1, H] col sums
    Q2 = 2.0 * PD - E * PD - S0
    C6 = (Q2 + PD) * NF + (K1 - K2) * NF + K1 - 2.0 * K2 + PD

    C5 = PDN - K2
    C7 = C6 / (NF + 2.0)
    NF2 = NF * NF
    NFp2 = NF + 2.0
    hx = np.empty((L, C_COLS), dtype=np.float32)
    hx[:, C_M1:C_M1 + H] = NF2 + NFp2 + OME
    hx[:, C_M0:C_M0 + H] = C5 * NF2 + C7 * NFp2 + C5 * OME
    hx[:, C_M2:C_M2 + H] = C5 * OME
    hx[:, C_OME:C_OME + H] = OME
    hx[:, C_NFP1:C_NFP1 + H] = NF + 1.0
    return {"t": t, "hx": hx}


def make_in_maps(s_edge, s_sib):
    return [_core_inputs(s_edge, s_sib, c) for c in range(N_CORES)]


def get_program():
    global _PROGRAM
    if _PROGRAM is None:
        _PROGRAM = _build_program()
    return _PROGRAM


def assemble(results):
    out = np.empty((4, L, L, 2), dtype=np.float32)
    for c in range(N_CORES):
        b, hs = c >> 1, (c & 1) * H
        out[b, :, hs:hs + H, :] = results[c]["o"].reshape(L, H, 2)
    return out


def kernel(s_edge, s_sib, mask):
    from concourse.bass_utils import run_bass_kernel_spmd

    s_edge = np.asarray(s_edge)
    s_sib = np.asarray(s_sib)
    mask = np.asarray(mask)
    assert mask.all(), "kernel specialized for the spec's all-ones mask"

    nc = get_program()
    in_maps = make_in_maps(s_edge, s_sib)
    res = run_bass_kernel_spmd(nc, in_maps, list(range(N_CORES))).results
    return assemble(res)


# revision 22
# speedup vs baseline: 1.0018x; 1.0018x over previous
"""Trainium2 Bass kernel for LoopyBeliefPropagation (3-iter, mask=ones).

Math: for each (b, h) slice define tile[d,s] = s_sib[b,d,h,s],
SP = softplus(tile).  Collapsing the reference's 3-iteration loop into
closed form (see kernel_baseline.py) and folding every
stream-independent term into host constants leaves, per (d,h):

  RS[d,h] = sum_s SP[d,s]        (row-reduce of the stream, DVE/Pool)
  CS[d,h] = sum_s SP[s,d]        (col sums via ones matmuls, PE)
  r1 = CS + C5
  w  = r1 * OME,  S1 = colsum(w) broadcast   (PE matmul)
  bdiff = r1*NF2 + (CS + C7)*NFp2 - RS*NFp1 + w - S1
  out1 = sigmoid(bdiff) = 1/(1+q),  q = exp(-max(bdiff, -30))
  out0 = q * out1

C5, C7, NF2, NFp1, NFp2, OME are [L,H] host constants (they only touch
O(L*H) gathered values, not the 4 MiB stream).  softplus = Ln(Exp(x)+1)
using the natural_log_exp ACT table; the +1 rides the Ln bias.

Device schedule: the s_sib shard streams in ramped h-chunks; Exp/Ln run
back-to-back on ACT (the spine; a dummy Exp right after a memset pulls
the ACT table load to t~0).  The h-columns are independent, so the tail
algebra runs in three h-slices pipelined against the spine; work is
spread over DVE and the otherwise-idle GPSIMD (Pool) engine so only the
last narrow slice (plus its output DMA) trails the spine.

Sharding: 8 cores x (b in 0..3, h-half in {0:64, 64:128}).
"""

import numpy as np

L = 128
H = 64            # h-slices per core
N_CORES = 8
LN2 = float(np.log(2.0))

# ramped chunk sizes; tails cover h [0:32], [32:58], [58:64]
CHUNKS = [2, 6, 10, 14, 16, 10, 4, 2]
TAILS = [(0, 32, 3), (32, 26, 5), (58, 6, 7)]    # (h0, width, after-chunk)
RED_ENG = ["v", "v", "v", "v", "v", "v", "v", "v"]   # per-chunk row-reduce
CHAIN_ENG = ["p", "p", "v"]                          # per-tail algebra
assert sum(CHUNKS) == H

# host-constant column layout: 5 tensors of H columns each
C_M1 = 0 * H
C_M0 = 1 * H
C_M2 = 2 * H
C_OME = 3 * H
C_NFP1 = 4 * H
C_COLS = 5 * H

_PROGRAM = None


def _build_program():
    import concourse.bacc as bacc
    import concourse.mybir as mybir
    import concourse.tile as tile

    fp32 = mybir.dt.float32
    AF = mybir.ActivationFunctionType
    OP = mybir.AluOpType

    # Exp and Ln live in one PWP table; without this filter the table
    # chooser maps Exp to exp_and_others and Ln to natural_log_exp_and_
    # others and reloads the ACT table (~1.3us) between every pair.
    if not getattr(bacc, "_lbp_act_tables_patched", False):
        _orig_tables = bacc.get_activation_tables

        def _ln_exp_only(arch):
            t = _orig_tables(arch)
            exp_ln = {AF.Exp, AF.Ln}
            return {
                name: (funcs if name == "natural_log_exp_and_others"
                       else set(funcs) - exp_ln)
                for name, funcs in t.items()
            }

        bacc.get_activation_tables = _ln_exp_only
        bacc._lbp_act_tables_patched = True

    nc = bacc.Bacc(None, target_bir_lowering=False)

    t_d = nc.dram_tensor("t", [L, H, L], fp32, kind="ExternalInput")
    hx_d = nc.dram_tensor("hx", [L, C_COLS], fp32, kind="ExternalInput")
    o_d = nc.dram_tensor("o", [L, H, 2], fp32, kind="ExternalOutput")

    with tile.TileContext(nc) as tc:
        with (
            tc.tile_pool(name="work", bufs=1) as wp,
            tc.tile_pool(name="psum", bufs=1, space="PSUM") as pp,
        ):
            zb = wp.tile([L, 1], fp32, tag="zb")
            ob = wp.tile([L, 1], fp32, tag="ob")
            ones = wp.tile([L, L], fp32, tag="ones")
            nc.gpsimd.memset(zb[:], 0.0)
            nc.gpsimd.memset(ob[:], 1.0)
            nc.gpsimd.memset(ones[:], 1.0)

            # dummy table-user: Bacc places the ACT table load right
            # before this, so it runs at t~0 instead of after chunk 0.
            dummy = wp.tile([L, 1], fp32, tag="dummy")
            nc.scalar.activation(dummy[:], zb[:], AF.Exp, bias=zb[:])

            # stream DMAs: chunk 0 first (critical-path opener); host
            # constants ride the queue behind chunk 3.
            hx = wp.tile([L, C_COLS], fp32, tag="hx")
            tchs = []
            h0 = 0
            for ci, ch in enumerate(CHUNKS):
                tch = wp.tile([L, ch, L], fp32, tag=f"tch{ci}")
                nc.sync.dma_start(tch[:], t_d[:, h0:h0 + ch, :])
                tchs.append((tch, h0, ch))
                if ci == 3:
                    nc.sync.dma_start(hx[:], hx_d[:])
                h0 += ch

            # per-tail state; tails 1 and 2 share merged exp/finals state
            BW = TAILS[1][1] + TAILS[2][1]
            bdcB = wp.tile([L, BW], fp32, tag="bdcB")
            qB = wp.tile([L, BW], fp32, tag="qB")
            sB = wp.tile([L, BW], fp32, tag="sB")
            osbB = wp.tile([L, BW, 2], fp32, tag="osbB")
            tails = []
            for ti, (th0, tw, _) in enumerate(TAILS):
                tails.append({
                    "RS": wp.tile([L, tw], fp32, tag=f"RS{ti}", name=f"RS{ti}"),
                    "cs": pp.tile([L, tw], fp32, tag=f"cs{ti}", name=f"cs{ti}"),
                    "bc": pp.tile([L, tw], fp32, tag=f"bc{ti}", name=f"bc{ti}"),
                    "osb": wp.tile([L, tw, 2], fp32, tag=f"osb{ti}",
                                   name=f"osb{ti}"),
                })
            tails[1]["bdcm"] = bdcB[:, 0:TAILS[1][1]]
            tails[2]["bdcm"] = bdcB[:, TAILS[1][1]:BW]

            def tail_of(h):
                for ti, (th0, tw, _) in enumerate(TAILS):
                    if th0 <= h < th0 + tw:
                        return ti, h - th0
                raise AssertionError

            def hxs(base, ti):
                th0, tw, _ = TAILS[ti]
                return hx[:, base + th0:base + th0 + tw]

            def tail_chain_pre(ti, eng, cs_src):
                """Algebra for tail slice ti up to bd-partial (pre -S1).

                cs_src: SBUF copy of CS for Pool (it cannot read PSUM);
                the PSUM-reading finish lives in tail_chain_fin.
                """
                th0, tw, _ = TAILS[ti]
                T = tails[ti]
                w_ = wp.tile([L, tw], fp32, tag=f"w_{ti}", name=f"w_{ti}")
                rsn = wp.tile([L, tw], fp32, tag=f"rsn_{ti}", name=f"rsn_{ti}")
                bd = wp.tile([L, tw], fp32, tag=f"bd_{ti}", name=f"bd_{ti}")
                T["bd"] = bd
                T["bdc"] = wp.tile([L, tw], fp32, tag=f"bdc_{ti}",
                                   name=f"bdc_{ti}")
                T["q"] = wp.tile([L, tw], fp32, tag=f"q_{ti}", name=f"q_{ti}")
                T["s"] = wp.tile([L, tw], fp32, tag=f"s_{ti}", name=f"s_{ti}")
                eng.tensor_mul(w_[:], cs_src[:], hxs(C_OME, ti))
                eng.tensor_add(w_[:], w_[:], hxs(C_M2, ti))
                nc.tensor.matmul(T["bc"][:], ones[:], w_[:],
                                 start=True, stop=True)
                eng.tensor_mul(bd[:], cs_src[:], hxs(C_M1, ti))
                eng.tensor_add(bd[:], bd[:], hxs(C_M0, ti))
                eng.tensor_mul(rsn[:], T["RS"][:], hxs(C_NFP1, ti))
                eng.tensor_sub(bd[:], bd[:], rsn[:])

            def tail_chain_fin(ti):
                """DVE finish: -S1 (PSUM read) and the exp-domain clamp."""
                T = tails[ti]
                dst = T["bdcm"] if "bdcm" in T else T["bdc"][:]
                nc.vector.tensor_sub(dst, T["bd"][:], T["bc"][:])
                nc.vector.tensor_scalar_max(dst, dst, -30.0)

            def tail_chain(ti, eng):
                if eng is nc.vector:
                    tail_chain_pre(ti, eng, tails[ti]["cs"])
                else:
                    th0, tw, _ = TAILS[ti]
                    cs_s = wp.tile([L, tw], fp32, tag=f"cs_s{ti}",
                                   name=f"cs_s{ti}")
                    nc.vector.tensor_copy(cs_s[:], tails[ti]["cs"][:])
                    tail_chain_pre(ti, eng, cs_s)

            def tail_exp(ti):
                T = tails[ti]
                nc.scalar.activation(T["q"][:], T["bdc"][:], AF.Exp,
                                     bias=zb[:], scale=-1.0)

            def tail_final(ti):
                T = tails[ti]
                nc.vector.tensor_scalar_add(T["s"][:], T["q"][:], 1.0)
                nc.vector.reciprocal(T["osb"][:, :, 1], T["s"][:])
                nc.vector.tensor_mul(T["osb"][:, :, 0], T["q"][:],
                                     T["osb"][:, :, 1])

            def tail_out(ti):
                th0, tw, _ = TAILS[ti]
                nc.sync.dma_start(o_d[:, th0:th0 + tw, :], tails[ti]["osb"][:])

            # spine: exp -> ln(+1) per chunk; row sums + col sums trail
            for ci, (tch, h0, ch) in enumerate(tchs):
                sp = wp.tile([L, ch, L], fp32, tag=f"sp{ci}", name=f"sp{ci}")
                if ci in (5, 6):
                    with tc.tile_wait_until({5: 0.0153, 6: 0.0184}[ci]):
                        nc.scalar.activation(sp[:], tch[:], AF.Exp, bias=zb[:])
                else:
                    nc.scalar.activation(sp[:], tch[:], AF.Exp, bias=zb[:])
                if ci == 6:
                    tail_exp(0)     # expA slots into the ACT queue here
                nc.scalar.activation(sp[:], sp[:], AF.Ln, bias=ob[:])
                ti, off = tail_of(h0)
                assert tail_of(h0 + ch - 1)[0] == ti
                T = tails[ti]
                red = nc.vector if RED_ENG[ci] == "v" else nc.gpsimd
                red.tensor_reduce(
                    T["RS"][:, off:off + ch], sp[:],
                    axis=mybir.AxisListType.X, op=OP.add,
                )
                for j in range(ch):
                    nc.tensor.matmul(
                        T["cs"][:, off + j:off + j + 1],
                        sp[:, j, :],
                        ones[:, 0:1],
                        start=True, stop=True,
                    )
                for ti2, (th0, tw, after) in enumerate(TAILS[:2]):
                    if after == ci:
                        eng = nc.gpsimd if CHAIN_ENG[ti2] == "p" else nc.vector
                        tail_chain(ti2, eng)
                if ci == 4:
                    tail_chain_fin(0)   # DVE: bd_A - S1_A, clamp
                if ci == 5:
                    tail_final(0)
                    tail_out(0)

            tail_chain(2, nc.gpsimd if CHAIN_ENG[2] == "p" else nc.vector)
            tail_chain_fin(2)
            tail_chain_fin(1)
            nc.scalar.activation(qB[:], bdcB[:], AF.Exp, bias=zb[:],
                                 scale=-1.0)
            nc.vector.tensor_scalar_add(sB[:], qB[:], 1.0)
            nc.vector.reciprocal(osbB[:, :, 1], sB[:])
            nc.vector.tensor_mul(osbB[:, :, 0], qB[:], osbB[:, :, 1])
            nc.sync.dma_start(o_d[:, TAILS[1][0]:H, :], osbB[:])

    nc.compile()
    return nc


def _softplus(x):
    return np.logaddexp(0.0, x)


def _core_inputs(s_edge, s_sib, c):
    b, hs = c >> 1, (c & 1) * H
    t = np.ascontiguousarray(s_sib[b, :, hs:hs + H, :], dtype=np.float32)
    d = np.arange(L)
    hl = np.arange(H)

    G = _softplus(t[d[:, None], hl[None, :], (hs + hl)[None, :]])
    DG = _softplus(t[d[:, None], hl[None, :], d[:, None]])
    ROWH = _softplus(
        s_sib[b, (hs + hl)[None, :], (hs + hl)[None, :], d[:, None]])

    E = (d[:, None] == (hs + hl)[None, :]).astype(np.float32)
    NF = 126.0 + E
    CN = LN2 * NF
    OME = 1.0 - E
    PD = (s_edge[b, :, hs:hs + H, 1]
          - s_edge[b, :, hs:hs + H, 0]).astype(np.float32)

    K1 = G + DG - E * G + CN
    K2 = ROWH + DG - E * DG + CN
    PDN = PD * (NF + 1.0)
    S0 = np.sum(PD * OME, axis=0, keepdims=True)   # [1, H] col sums
    Q2 = 2.0 * PD - E * PD - S0
    C6 = (Q2 + PD) * NF + (K1 - K2) * NF + K1 - 2.0 * K2 + PD

    C5 = PDN - K2
    C7 = C6 / (NF + 2.0)
    NF2 = NF * NF
    NFp2 = NF + 2.0
    hx = np.empty((L, C_COLS), dtype=np.float32)
    hx[:, C_M1:C_M1 + H] = NF2 + NFp2 + OME
    hx[:, C_M0:C_M0 + H] = C5 * NF2 + C7 * NFp2 + C5 * OME
    hx[:, C_M2:C_M2 + H] = C5 * OME
    hx[:, C_OME:C_OME + H] = OME
    hx[:, C_NFP1:C_NFP1 + H] = NF + 1.0
    return {"t": t, "hx": hx}


def make_in_maps(s_edge, s_sib):
    return [_core_inputs(s_edge, s_sib, c) for c in range(N_CORES)]


def get_program():
    global _PROGRAM
    if _PROGRAM is None:
        _PROGRAM = _build_program()
    return _PROGRAM


def assemble(results):
    out = np.empty((4, L, L, 2), dtype=np.float32)
    for c in range(N_CORES):
        b, hs = c >> 1, (c & 1) * H
        out[b, :, hs:hs + H, :] = results[c]["o"].reshape(L, H, 2)
    return out


def kernel(s_edge, s_sib, mask):
    from concourse.bass_utils import run_bass_kernel_spmd

    s_edge = np.asarray(s_edge)
    s_sib = np.asarray(s_sib)
    mask = np.asarray(mask)
    assert mask.all(), "kernel specialized for the spec's all-ones mask"

    nc = get_program()
    in_maps = make_in_maps(s_edge, s_sib)
    res = run_bass_kernel_spmd(nc, in_maps, list(range(N_CORES))).results
    return assemble(res)
